# revision 1
# baseline (speedup 1.0000x reference)
"""Batch depthwise cross-correlation on 8 Trainium2 NeuronCores.

Problem: x [8, 256, 64, 64] f32, templates [8, 8, 256, 7, 7] f32
         out[t, b, c, i, j] = sum_{u,v} xpad[b, c, i+u, j+v] * templates[t, b, c, u, v]
         (7x7 'same' cross-correlation, depthwise over (b, c), vmapped over t)

Sharding: by batch b -> core b. Each core computes all 8 templates for its
batch; the per-batch image patches are shared by all 8 templates.

Device kernel (TensorEngine): per channel the conv is 2 dense matmuls.
Host pre-tiles the padded image into overlapping 8x14 patches at stride
(2, 8): im2colT[k=(di,dj)=112, c, n=(ti,tj)=256] bf16, and expands each
channel's 8 templates into a dense [112, (t,oi,oj)=128] bf16 block
(wexp[(di,dj),(t,oi,oj)] = w[t,di-oi,dj-oj]). On device, matmul g (g=0,1)
takes patches tj = 2*tjq + g as the stationary operand (m=(ti,tjq)=128)
and streams all weight columns (n=128), accumulating fp32 in PSUM cols
[128g:128g+128]. A contiguous vector-engine copy drains PSUM -> SBUF and
one contiguous DMA stores the [128, 256] block to a per-channel scratch
layout in DRAM; the host unscrambles to [t, c, i, j] at the end
(i = 2*ti + oi, j = 8*(2*tjq + g) + oj).

DMA structure (queue-issue cost and block size dominate): patch and weight
loads stream 16 channels per DMA (SP / ACT queues); results stage 8
channels in SBUF and store as one [128, 8*256] DMA with 8 KiB contiguous
blocks into a p-major DRAM scratch tensor, alternating SP/ACT queues;
PSUM drains all run on the vector engine. Measured ~160-175 us per core
(~370-400 GB/s sustained DMA, the fabric roofline for the ~55 MB moved).
"""

import numpy as np
import ml_dtypes

import concourse.bacc as bacc
import concourse.mybir as mybir
from concourse.tile import TileContext
from concourse import bass_utils

F32 = mybir.dt.float32
BF16 = mybir.dt.bfloat16

N_CORES = 8
BS = 8
NT = 8
NC_CH = 256
HI = WI = 64
PAD = 3
PH, PW = 70, 70  # padded image (host-side only)
PR, PC = 8, 14  # patch rows x cols
SR, SC = 2, 8  # patch strides
KP = PR * PC  # 112 = contraction (di, dj)
NPATCH = 256  # (ti, tj) = 32 * 8
NW = NT * SR * SC  # 128 = (t, oi, oj) weight columns
CB = 16  # channels per input DMA

_prog_cache = {}


def _build_program():
    nc = bacc.Bacc("TRN2", debug=False, target_bir_lowering=False, num_devices=N_CORES)

    xt = nc.dram_tensor("xt", [KP, NC_CH * NPATCH], BF16, kind="ExternalInput").ap()
    wt = nc.dram_tensor("wt", [KP, NC_CH * NW], BF16, kind="ExternalInput").ap()
    # scratch-layout output (p-major for big contiguous DMA blocks); host unscrambles
    out = nc.dram_tensor("out", [128, NC_CH, 2 * NW], F32, kind="ExternalOutput").ap()

    with TileContext(nc) as tc:
        with (
            tc.tile_pool(name="wpool", bufs=4) as wpool,
            tc.tile_pool(name="xpool", bufs=4) as xpool,
            tc.tile_pool(name="psum", bufs=8, space="PSUM") as ppool,
            tc.tile_pool(name="opool", bufs=4) as opool,
        ):
            for c0 in range(0, NC_CH, CB):
                xs = xpool.tile([KP, CB * NPATCH], BF16, tag="xs")
                nc.sync.dma_start(
                    out=xs, in_=xt[:, c0 * NPATCH : (c0 + CB) * NPATCH]
                )
                ws = wpool.tile([KP, CB * NW], BF16, tag="ws")
                nc.scalar.dma_start(out=ws, in_=wt[:, c0 * NW : (c0 + CB) * NW])
                ws_v = ws.rearrange("k (c f) -> k c f", c=CB)
                xs_v = xs.rearrange("k (c ti tjq g) -> k c ti tjq g", c=CB, ti=32, g=2)
                for ci in range(CB):
                    c = c0 + ci
                    ps = ppool.tile([128, 2 * NW], F32, tag="ps")
                    for g in range(2):
                        nc.tensor.matmul(
                            out=ps[:, g * NW : (g + 1) * NW],
                            lhsT=xs_v[:, ci, :, :, g],
                            rhs=ws_v[:, ci],
                        )
                    if ci % 8 == 0:
                        os_ = opool.tile([128, 8 * 2 * NW], F32, tag="os")
                        os_v = os_.rearrange("p (c f) -> p c f", c=8)
                    nc.vector.tensor_copy(out=os_v[:, ci % 8], in_=ps)
                    if ci % 8 == 7:
                        dma_eng = nc.sync if (c // 8) % 2 == 0 else nc.scalar
                        dma_eng.dma_start(out=out[:, c - 7 : c + 1], in_=os_)
    nc.compile()
    return nc


def _get_program():
    if "nc" not in _prog_cache:
        _prog_cache["nc"] = _build_program()
    return _prog_cache["nc"]


def _host_prep(x, templates):
    """Build per-core im2colT patches and expanded weight blocks."""
    xpad = np.zeros((BS, NC_CH, PH, PW), np.float32)
    xpad[:, :, PAD : PAD + HI, PAD : PAD + WI] = x
    # windows [b, c, ti, tj, di, dj]
    v = np.lib.stride_tricks.sliding_window_view(xpad, (PR, PC), axis=(2, 3))
    v = v[:, :, :: SR, :: SC]  # [b, c, 32, 8, 8, 14]
    # -> [b, (di,dj)=112, c, (ti, tjq, g)=256] with tj = 2*tjq + g
    v = v.reshape(BS, NC_CH, 32, 4, 2, PR, PC)  # ti, tjq, g, di, dj
    im2colT = np.ascontiguousarray(
        v.transpose(0, 5, 6, 1, 2, 3, 4).reshape(BS, KP, NC_CH * NPATCH)
    ).astype(ml_dtypes.bfloat16)

    # wexp[b, di, dj, c, t, oi, oj] = templates[t, b, c, di-oi, dj-oj]
    wexp = np.zeros((BS, PR, PC, NC_CH, NT, SR, SC), np.float32)
    w_t = templates.transpose(1, 3, 4, 2, 0)  # [b, u, v, c, t]
    for oi in range(SR):
        for oj in range(SC):
            wexp[:, oi : oi + 7, oj : oj + 7, :, :, oi, oj] = w_t
    wexp = np.ascontiguousarray(wexp.reshape(BS, KP, NC_CH * NW)).astype(
        ml_dtypes.bfloat16
    )
    return im2colT, wexp


def _unscramble(res):
    """[128, 256, 256] scratch -> [8, 256, 64, 64]."""
    v = res.reshape(32, 4, NC_CH, 2, NT, SR, SC)  # ti, tjq, c, g, t, oi, oj
    # out[t, c, i=(ti,oi), j=(tjq,g,oj)]
    v = v.transpose(4, 2, 0, 5, 1, 3, 6)  # t, c, ti, oi, tjq, g, oj
    return np.ascontiguousarray(v.reshape(NT, NC_CH, HI, WI))


def kernel(x, templates):
    x = np.asarray(x, dtype=np.float32)
    templates = np.asarray(templates, dtype=np.float32)

    im2colT, wexp = _host_prep(x, templates)

    nc = _get_program()
    in_maps = [{"xt": im2colT[b], "wt": wexp[b]} for b in range(BS)]
    res = bass_utils.run_bass_kernel_spmd(nc, in_maps, list(range(N_CORES))).results
    return np.stack([_unscramble(res[b]["out"]) for b in range(BS)], axis=1)



# revision 2
# speedup vs baseline: 1.1612x; 1.1612x over previous
"""Batch depthwise cross-correlation on 8 Trainium2 NeuronCores.

Problem: x [8, 256, 64, 64] f32, templates [8, 8, 256, 7, 7] f32
         out[t, b, c, i, j] = sum_{u,v} xpad[b, c, i+u, j+v] * templates[t, b, c, u, v]
         (7x7 'same' cross-correlation, depthwise over (b, c), vmapped over t)

Sharding: by batch b -> core b. Each core computes all 8 templates for its
batch; the per-batch image patches are shared by all 8 templates.

Device kernel (TensorEngine): per channel the conv is 1 dense matmul with
stationary weights. Host pre-tiles the padded image into overlapping 8x14
patches at stride (2, 8): im2colT[k=(di,dj)=112, c, n=(ti,tjq,g)=256] bf16,
and expands each channel's 8 templates into a dense [112, (t,oi,oj)=128]
bf16 block (wexp[(di,dj),(t,oi,oj)] = w[t,di-oi,dj-oj]). On device, per
channel: LDWEIGHTS wexp [112,128] (stationary), one MATMUL streaming all
256 patch columns, accumulating fp32 in PSUM [128=(t,oi,oj), 256 patches].
Drains convert PSUM f32 -> SBUF bf16 (alternating Vector/Activation
engines), staged 16 channels per output DMA; host upcasts + unscrambles
(i = 2*ti + oi, j = 8*(2*tjq + g) + oj).
"""

import numpy as np
import ml_dtypes

import concourse.bacc as bacc
import concourse.mybir as mybir
from concourse.tile import TileContext
from concourse import bass_utils

F32 = mybir.dt.float32
BF16 = mybir.dt.bfloat16

N_CORES = 8
BS = 8
NT = 8
NC_CH = 256
HI = WI = 64
PAD = 3
PH, PW = 70, 70  # padded image (host-side only)
PR, PC = 8, 14  # patch rows x cols
SR, SC = 2, 8  # patch strides
KP = PR * PC  # 112 = contraction (di, dj)
NPATCH = 256  # (ti, tjq, g) = 32 * 4 * 2
NW = NT * SR * SC  # 128 = (t, oi, oj) weight columns
CB = 16  # channels per input DMA / output staging block

_prog_cache = {}


def _build_program():
    nc = bacc.Bacc("TRN2", debug=False, target_bir_lowering=False, num_devices=N_CORES)

    xt = nc.dram_tensor("xt", [KP, NC_CH * NPATCH], BF16, kind="ExternalInput").ap()
    wt = nc.dram_tensor("wt", [KP, NC_CH * NW], BF16, kind="ExternalInput").ap()
    # scratch-layout output (p-major for big contiguous DMA blocks); host unscrambles
    out = nc.dram_tensor("out", [NW, NC_CH, NPATCH], BF16, kind="ExternalOutput").ap()

    with TileContext(nc) as tc:
        with (
            tc.tile_pool(name="wpool", bufs=3) as wpool,
            tc.tile_pool(name="xpool", bufs=3) as xpool,
            tc.tile_pool(name="psum", bufs=8, space="PSUM") as ppool,
            tc.tile_pool(name="opool", bufs=2) as opool,
        ):
            for c0 in range(0, NC_CH, CB):
                xs = xpool.tile([KP, CB * NPATCH], BF16, tag="xs")
                nc.sync.dma_start(
                    out=xs, in_=xt[:, c0 * NPATCH : (c0 + CB) * NPATCH]
                )
                ws = wpool.tile([KP, CB * NW], BF16, tag="ws")
                nc.sync.dma_start(out=ws, in_=wt[:, c0 * NW : (c0 + CB) * NW])
                ws_v = ws.rearrange("k (c f) -> k c f", c=CB)
                xs_v = xs.rearrange("k (c f) -> k c f", c=CB)
                os_ = opool.tile([NW, CB * NPATCH], BF16, tag="os")
                os_v = os_.rearrange("p (c f) -> p c f", c=CB)
                for ci in range(CB):
                    ps = ppool.tile([NW, NPATCH], F32, tag="ps")
                    nc.tensor.matmul(out=ps, lhsT=ws_v[:, ci], rhs=xs_v[:, ci])
                    if ci % 2 == 0:
                        nc.vector.tensor_copy(out=os_v[:, ci], in_=ps)
                    else:
                        nc.scalar.copy(out=os_v[:, ci], in_=ps)
                nc.gpsimd.dma_start(out=out[:, c0 : c0 + CB], in_=os_)
    nc.compile()
    return nc


def _get_program():
    if "nc" not in _prog_cache:
        _prog_cache["nc"] = _build_program()
    return _prog_cache["nc"]


def _host_prep(x, templates):
    """Build per-core im2colT patches and expanded weight blocks."""
    xpad = np.zeros((BS, NC_CH, PH, PW), np.float32)
    xpad[:, :, PAD : PAD + HI, PAD : PAD + WI] = x
    # windows [b, c, ti, tj, di, dj]
    v = np.lib.stride_tricks.sliding_window_view(xpad, (PR, PC), axis=(2, 3))
    v = v[:, :, :: SR, :: SC]  # [b, c, 32, 8, 8, 14]
    # -> [b, (di,dj)=112, c, (ti, tjq, g)=256] with tj = 2*tjq + g
    v = v.reshape(BS, NC_CH, 32, 4, 2, PR, PC)  # ti, tjq, g, di, dj
    im2colT = np.ascontiguousarray(
        v.transpose(0, 5, 6, 1, 2, 3, 4).reshape(BS, KP, NC_CH * NPATCH)
    ).astype(ml_dtypes.bfloat16)

    # wexp[b, di, dj, c, t, oi, oj] = templates[t, b, c, di-oi, dj-oj]
    wexp = np.zeros((BS, PR, PC, NC_CH, NT, SR, SC), np.float32)
    w_t = templates.transpose(1, 3, 4, 2, 0)  # [b, u, v, c, t]
    for oi in range(SR):
        for oj in range(SC):
            wexp[:, oi : oi + 7, oj : oj + 7, :, :, oi, oj] = w_t
    wexp = np.ascontiguousarray(wexp.reshape(BS, KP, NC_CH * NW)).astype(
        ml_dtypes.bfloat16
    )
    return im2colT, wexp


def _unscramble(res):
    """[128=(t,oi,oj), 256=c, 256=(ti,tjq,g)] bf16 scratch -> [8, 256, 64, 64] f32."""
    v = res.astype(np.float32).reshape(NT, SR, SC, NC_CH, 32, 4, 2)
    # out[t, c, i=(ti,oi), j=(tjq,g,oj)]
    v = v.transpose(0, 3, 4, 1, 5, 6, 2)  # t, c, ti, oi, tjq, g, oj
    return np.ascontiguousarray(v.reshape(NT, NC_CH, HI, WI))


def kernel(x, templates):
    x = np.asarray(x, dtype=np.float32)
    templates = np.asarray(templates, dtype=np.float32)

    im2colT, wexp = _host_prep(x, templates)

    nc = _get_program()
    in_maps = [{"xt": im2colT[b], "wt": wexp[b]} for b in range(BS)]
    res = bass_utils.run_bass_kernel_spmd(nc, in_maps, list(range(N_CORES))).results
    return np.stack([_unscramble(res[b]["out"]) for b in range(BS)], axis=1)
